# revision 43
# baseline (speedup 1.0000x reference)
"""Trainium2 Bass kernel for nn_HausdorffDistance_28406913696124.

Math (reference):
    px = (prob_map[0].ravel() >= 0.5)                 # [N], N = 100*100
    py = (gt_map.ravel()   >= 0.5)                    # [N]
    D[i,j] = euclid dist between grid points i, j     # [N, N] constant!
    loss   = mean_i | px_i * mean_j D[i,j] - (D @ py)_i / N |

Key structure: with i = (r, c), j = (a, b) on the 100x100 grid,
(D @ py)[r,c] = sum_{a,b} PY[a,b] * K[r-a, c-b] with the 199x199 kernel
K[u,v] = sqrt(u^2 + v^2).  K is smooth, so its (pair-count weighted) SVD
truncates hard: rank 3 gives ~1e-6 relative error on the final scalar
(tolerance is 2e-2).  With K ~= sum_s f_s (x) g_s the correlation
factorizes into per-rank symmetric-Toeplitz sandwiches

    a := D@py ~= sum_s F_s @ PY @ G_s,  F_s[r,a] = f_s[r-a], G_s[b,c] = g_s[c-b]

i.e. 2*RANK = 6 matmuls of [100,100] @ [100,100] TOTAL -- small enough
that a SINGLE core beats any multi-core split (an 8-core collective
alone has a ~5us floor).  The program runs replicated on all 8 cores
(SPMD, identical inputs); core 0's output is returned.

Device pipeline (one core), b := px * rowsum (rowsum precomputed, it is
input independent), both a, b >= 0:

    sum_i |a_i - b_i| = sum(a) + sum(b) - 2*sum(min(a, b))

  stage1  Z_s[b,r] = sum_a PY[a,b] F_s[a,r]      (lhsT=PY stationary)
  copies  Z: PSUM -> SBUF fp16 on DVE/ACT/Pool in parallel, each with a
          fused accum_out giving zrow_s[b] = sum_r Z_s[b,r]
  stage2  acc[r,c] += sum_b Z_s[b,r] G_s[b,c]    (PSUM accumulate)
  sum(a)  = sum_b zrow_s[b] * gcol_s[b] summed over s (gcol precomputed)
          -- no extra pass over acc needed; fused with sum(b) (the pxrN
          accum row-sums land in zrowcat's 4th column, gcol's 4th column
          is -0.5) into ONE dot-with-accum op
  -2min   via max(-2a, -2b) = -2min(a,b): ONE DVE pass over acc with
          accum_out (pxrN = -2e-8*b uses a host-scaled rowsum table; the
          1e-8 = 1/N^2 output scale is folded into every column)
  finale  [100,2] partial columns -> DVE free-dim reduce -> Pool
          partition_all_reduce (GPSIMD ucode) -> scalar -> DMA out.

Inputs are packed into three byte blobs, one per DMA queue (SP/ACT
HWDGE + Pool SWDGE, which runs on a separate descriptor-generation
path): the ~2.2us fixed DMA latency dominates transfer time at these
sizes, and the critical consumer of each blob differs.  Dummy PE
matmuls from t=0 keep the tensor engine's p-state ramped.
"""

import sys

import numpy as np

sys.path.insert(0, "/opt/trn_rl_repo")

H = 100
N = H * H
NCORES = 8
RANK = 3
FW = RANK * H              # stacked factor width (300)
HOT_B = 4 * H + 2 * FW     # gt f32 | Fstack f16            = 1000 B/row
GCOLD_B = 2 * FW + 4 * (RANK + 1)  # Gstack f16 | gcolcat f32 = 1216 B/row
PCOLD_B = 8 * H            # prob f32 | rowsumN2 f32        =  800 B/row

# PE p-state warmup: free-dim sizes of dummy matmuls issued from t=0.
WARMUP = [256] * 10


def _host_constants():
    """Geometry-only constant tables (input independent)."""
    idx = np.arange(H)
    # rowsum[r,c] = sum_j D[i,j] (i = r*100+c), accumulated in float64.
    absdiff = np.abs(idx[:, None] - idx[None, :])
    q64 = np.sqrt((idx[:, None] ** 2 + idx[None, :] ** 2).astype(np.float64))
    cnt = np.zeros((H, H))
    np.add.at(cnt, (idx[:, None], absdiff), 1.0)
    rowsum = cnt @ q64 @ cnt.T
    # pxrN = (prob>=.5) * rowsumN2 = -2e-8*b for the max trick; the 1e-8
    # (= 1/N^2) output scale is folded into all partial-sum columns so the
    # final step is a plain all-reduce.
    rowsumn2 = (-2.0e-8 * rowsum).astype(np.float32)

    # Pair-count weighted SVD of K[u,v] = sqrt(u^2+v^2), u,v in [-99,99].
    u = np.arange(-(H - 1), H)
    K = np.sqrt((u[:, None] ** 2 + u[None, :] ** 2).astype(np.float64))
    w_cnt = (H - np.abs(u)).astype(np.float64)
    sw = np.sqrt(w_cnt)
    U, S, Vt = np.linalg.svd(sw[:, None] * K * sw[None, :])

    toep = (idx[:, None] - idx[None, :]) + (H - 1)
    fstack = np.empty((H, FW), dtype=np.float16)
    gstack = np.empty((H, FW), dtype=np.float16)
    for s in range(RANK):
        f = (U[:, s] * np.sqrt(S[s])) / sw
        g = (Vt[s, :] * np.sqrt(S[s])) / sw
        # F_s[a, r] = f_s[r-a]; G_s[b, c] = g_s[c-b]  (both symmetric)
        fstack[:, s * H:(s + 1) * H] = f[toep].astype(np.float16)
        gstack[:, s * H:(s + 1) * H] = g[toep].astype(np.float16)
    # gcol_s[b] = 1e-8 * sum_c G16_s[b, c] (sum the f16-rounded table in
    # f64 so the separable sum(a) matches the on-device acc; f32 storage
    # since the 1e-8 prescale would underflow f16)
    gcolcat = np.stack(
        [gstack[:, s * H:(s + 1) * H].astype(np.float64).sum(1)
         for s in range(RANK)], axis=1
    )
    # 4th column: -0.5, turning the fused dot with zrowcat (whose 4th
    # column is the accum of pxrN = -2e-8*b rows) into +1e-8*sum(b)
    gcolcat = np.concatenate(
        [1e-8 * gcolcat, np.full((H, 1), -0.5)], axis=1
    ).astype(np.float32)  # [100, RANK+1]
    return rowsumn2, fstack, gstack, gcolcat


def _build_module():
    import concourse.bacc as bacc
    import concourse.mybir as mybir
    import concourse.tile as tile

    f32 = mybir.dt.float32
    f16 = mybir.dt.float16
    u8 = mybir.dt.uint8
    Alu = mybir.AluOpType

    nc = bacc.Bacc(
        "TRN2",
        target_bir_lowering=False,
        debug=False,
        enable_asserts=False,
        num_devices=NCORES,
    )

    hot_d = nc.dram_tensor("hot", [H, HOT_B], u8, kind="ExternalInput")
    gcold_d = nc.dram_tensor("gcold", [H, GCOLD_B], u8, kind="ExternalInput")
    pcold_d = nc.dram_tensor("pcold", [H, PCOLD_B], u8, kind="ExternalInput")
    out_d = nc.dram_tensor("out", [1, 1], f32, kind="ExternalOutput")

    with tile.TileContext(nc) as tc:
        with (
            tc.tile_pool(name="sb", bufs=1) as sb,
            tc.tile_pool(name="ps_z0", bufs=1, space="PSUM") as ps_z0,
            tc.tile_pool(name="ps_z1", bufs=1, space="PSUM") as ps_z1,
            tc.tile_pool(name="ps_z2", bufs=1, space="PSUM") as ps_z2,
            tc.tile_pool(name="ps_acc", bufs=1, space="PSUM") as ps_acc,
            tc.tile_pool(name="ps_w", bufs=1, space="PSUM") as ps_w,
        ):
            # ---- t=0: three input DMAs on three descriptor queues -------
            # (hot + pcold share SP's HWDGE back-to-back; gcold goes via
            # Pool's SWDGE which generates descriptors on a separate path)
            hot_sb = sb.tile([H, HOT_B], u8)
            nc.sync.dma_start(hot_sb[:], hot_d[:])
            pcold_sb = sb.tile([H, PCOLD_B], u8)
            nc.sync.dma_start(pcold_sb[:], pcold_d[:])
            gcold_sb = sb.tile([H, GCOLD_B], u8)
            nc.gpsimd.dma_start(gcold_sb[:], gcold_d[:])

            gt_v = hot_sb[:, 0:4 * H].bitcast(f32)
            f_v = hot_sb[:, 4 * H:HOT_B].bitcast(f16)
            prob_v = pcold_sb[:, 0:4 * H].bitcast(f32)
            rowsumn2_v = pcold_sb[:, 4 * H:PCOLD_B].bitcast(f32)
            g_v = gcold_sb[:, 0:2 * FW].bitcast(f16)
            gcol_v = gcold_sb[:, 2 * FW:GCOLD_B].bitcast(f32)

            # ---- t=0: constants + PE p-state warmup ---------------------
            wz = sb.tile([H, 256], f16)
            nc.vector.memset(wz[:], 0.0)
            wps = ps_w.tile([H, 256], f32)
            for wn in WARMUP:
                nc.tensor.matmul(
                    wps[:, 0:wn], wz[:, 0:H], wz[:, 0:wn], start=True, stop=True
                )

            # ---- hot path: binarize -> stage1 -> copy -> stage2 ---------
            py16 = sb.tile([H, H], f16)
            nc.vector.tensor_scalar(py16[:], gt_v, 0.5, None, Alu.is_ge)

            zps0 = ps_z0.tile([H, H], f32)
            zps1 = ps_z1.tile([H, H], f32)
            zps2 = ps_z2.tile([H, H], f32)
            for s, zps in enumerate((zps0, zps1, zps2)):
                sl = slice(s * H, (s + 1) * H)
                nc.tensor.matmul(
                    zps[:], py16[:], f_v[:, sl], start=True, stop=True
                )

            # PSUM -> SBUF (fp16) copies, DVE + ACT + DVE (Pool is GPSIMD
            # and cannot read PSUM), DVE ones with a fused per-partition
            # row-sum (zrow_s[b]).
            # (separate tiles: the dep tracker serializes same-tile access)
            zsb0 = sb.tile([H, H], f16)
            zsb1 = sb.tile([H, H], f16)
            zsb2 = sb.tile([H, H], f16)
            # zrow_s[b] = sum_r Z_s[b,r], all three into one DVE-written
            # tile so the downstream product is a single fused op; the 4th
            # column takes the pxrN row-sums (see gcolcat's -0.5 column)
            zrowcat = sb.tile([H, RANK + 1], f32)
            nc.vector.tensor_scalar(
                zsb0[:], zps0[:], 0.0, 0.0, Alu.add, Alu.add,
                accum_out=zrowcat[:, 0:1],
            )
            nc.scalar.copy(zsb1[:], zps1[:])
            nc.vector.tensor_scalar(
                zsb2[:], zps2[:], 0.0, 0.0, Alu.add, Alu.add,
                accum_out=zrowcat[:, 2:3],
            )
            # zrow1 via a cheap 16-bit re-read on DVE (ACT's fused
            # accumulator read would cost a serial 187ns on the zsb1 path)
            junkz = sb.tile([H, H], f16)
            with tc.tile_wait_until(0.0034):
                nc.vector.tensor_scalar(
                    junkz[:], zsb1[:], 0.0, 0.0, Alu.add, Alu.add,
                    accum_out=zrowcat[:, 1:2],
                )

            acc = ps_acc.tile([H, H], f32)
            for s, zsb in enumerate((zsb0, zsb1, zsb2)):
                sl = slice(s * H, (s + 1) * H)
                nc.tensor.matmul(
                    acc[:], zsb[:], g_v[:, sl],
                    start=(s == 0), stop=(s == RANK - 1),
                )

            # ---- tail: partial-sum columns [100, 2], all on DVE ---------
            # (all at 1e-8 = 1/N^2 scale so the finale is a plain reduce;
            # single-engine writers avoid cross-engine same-tile chains)
            # c0: sum(a) + sum(b) rows, one fused dot of zrowcat (zrows +
            #     pxrN row-sums) with gcolcat (1e-8*gcol_s | -0.5)
            # c1: -2e-8*min(a,b) via max(-2e-8*a, pxrN)
            # Pool is GPSIMD: no PSUM access, no general elementwise ops.
            # wait_until hints keep the scheduler from queueing the tail
            # ahead of the critical Z-copies on DVE's in-order queue.
            cols = sb.tile([H, 2], f32)
            pxrn = sb.tile([H, H], f32)
            junkm = sb.tile([H, H], f32)
            with tc.tile_wait_until(0.0035):
                nc.vector.scalar_tensor_tensor(
                    pxrn[:], prob_v, 0.5, rowsumn2_v,
                    op0=Alu.is_ge, op1=Alu.mult,
                    accum_out=zrowcat[:, RANK:RANK + 1],
                )
            junkp = sb.tile([H, RANK + 1], f32)
            with tc.tile_wait_until(0.0037):
                nc.vector.scalar_tensor_tensor(
                    junkp[:], zrowcat[:], 0.0, gcol_v[:],
                    op0=Alu.add, op1=Alu.mult, accum_out=cols[:, 0:1],
                )
            with tc.tile_wait_until(0.0044):
                nc.vector.scalar_tensor_tensor(
                    junkm[:], acc[:], -2.0e-8, pxrn[:],
                    op0=Alu.mult, op1=Alu.max, accum_out=cols[:, 1:2],
                )

            # ---- finale: free-dim reduce (DVE) + cross-partition --------
            # all-reduce (Pool ucode) + store
            colsum = sb.tile([H, 1], f32)
            with tc.tile_wait_until(0.0045):
                nc.vector.tensor_reduce(
                    colsum[:], cols[:], axis=mybir.AxisListType.X, op=Alu.add
                )
            from concourse import bass_isa
            res = sb.tile([H, 1], f32)
            nc.gpsimd.partition_all_reduce(
                res[:], colsum[:], channels=H, reduce_op=bass_isa.ReduceOp.add
            )
            nc.sync.dma_start(out_d[:], res[0:1, 0:1])

    nc.compile()
    return nc


_STATE = {}


def _get_state():
    if not _STATE:
        rowsumn2, fstack, gstack, gcolcat = _host_constants()
        _STATE["consts"] = (rowsumn2, fstack, gstack, gcolcat)
        _STATE["nc"] = _build_module()
    return _STATE


def _in_maps(prob_map, gt_map):
    st = _get_state()
    rowsumn2, fstack, gstack, gcolcat = st["consts"]
    prob = np.asarray(prob_map, dtype=np.float32).reshape(H, H)
    gt = np.asarray(gt_map, dtype=np.float32).reshape(H, H)

    hot = np.ascontiguousarray(np.concatenate(
        [gt.view(np.uint8), fstack.view(np.uint8)], axis=1
    ))
    gcold = np.ascontiguousarray(np.concatenate(
        [gstack.view(np.uint8), gcolcat.view(np.uint8)], axis=1
    ))
    pcold = np.ascontiguousarray(np.concatenate(
        [prob.view(np.uint8), rowsumn2.view(np.uint8)], axis=1
    ))
    return [
        {"hot": hot, "gcold": gcold, "pcold": pcold} for _ in range(NCORES)
    ]


def _run(prob_map, gt_map, trace=False, **spmd_kwargs):
    from concourse import bass_utils

    st = _get_state()
    in_maps = _in_maps(prob_map, gt_map)
    res = bass_utils.run_bass_kernel_spmd(
        st["nc"], in_maps, core_ids=list(range(NCORES)), trace=trace,
        **spmd_kwargs,
    )
    value = np.float32(res.results[0]["out"][0, 0])
    return value, res


def kernel(prob_map, gt_map):
    value, _ = _run(prob_map, gt_map, trace=False)
    return np.asarray(value, dtype=np.float32)


# revision 44
# speedup vs baseline: 1.0201x; 1.0201x over previous
"""Trainium2 Bass kernel for nn_HausdorffDistance_28406913696124.

Math (reference):
    px = (prob_map[0].ravel() >= 0.5)                 # [N], N = 100*100
    py = (gt_map.ravel()   >= 0.5)                    # [N]
    D[i,j] = euclid dist between grid points i, j     # [N, N] constant!
    loss   = mean_i | px_i * mean_j D[i,j] - (D @ py)_i / N |

Key structure: with i = (r, c), j = (a, b) on the 100x100 grid,
(D @ py)[r,c] = sum_{a,b} PY[a,b] * K[r-a, c-b] with the 199x199 kernel
K[u,v] = sqrt(u^2 + v^2).  K is smooth, so its (pair-count weighted) SVD
truncates hard: rank 3 gives ~1e-6 relative error on the final scalar
(tolerance is 2e-2).  With K ~= sum_s f_s (x) g_s the correlation
factorizes into per-rank symmetric-Toeplitz sandwiches

    a := D@py ~= sum_s F_s @ PY @ G_s,  F_s[r,a] = f_s[r-a], G_s[b,c] = g_s[c-b]

i.e. 2*RANK = 6 matmuls of [100,100] @ [100,100] TOTAL -- small enough
that a SINGLE core beats any multi-core split (an 8-core collective
alone has a ~5us floor).  The program runs replicated on all 8 cores
(SPMD, identical inputs); core 0's output is returned.

Device pipeline (one core), b := px * rowsum (rowsum precomputed, it is
input independent), both a, b >= 0:

    sum_i |a_i - b_i| = sum(a) + sum(b) - 2*sum(min(a, b))

  stage1  Z_s[b,r] = sum_a PY[a,b] F_s[a,r]      (lhsT=PY stationary)
  copies  Z: PSUM -> SBUF fp16 on DVE/ACT/Pool in parallel, each with a
          fused accum_out giving zrow_s[b] = sum_r Z_s[b,r]
  stage2  acc[r,c] += sum_b Z_s[b,r] G_s[b,c]    (PSUM accumulate)
  sum(a)  = sum_b zrow_s[b] * gcol_s[b] summed over s (gcol precomputed)
          -- no extra pass over acc needed; fused with sum(b) (the pxrN
          accum row-sums land in zrowcat's 4th column, gcol's 4th column
          is -0.5) into ONE dot-with-accum op
  -2min   via max(-2a, -2b) = -2min(a,b): ONE DVE pass over acc with
          accum_out (pxrN = -2e-8*b uses a host-scaled rowsum table; the
          1e-8 = 1/N^2 output scale is folded into every column)
  finale  [100,2] partial columns -> DVE free-dim reduce -> Pool
          partition_all_reduce (GPSIMD ucode) -> scalar -> DMA out.

Inputs are packed into three byte blobs, one per DMA queue (SP/ACT
HWDGE + Pool SWDGE, which runs on a separate descriptor-generation
path): the ~2.2us fixed DMA latency dominates transfer time at these
sizes, and the critical consumer of each blob differs.  Dummy PE
matmuls from t=0 keep the tensor engine's p-state ramped.
"""

import sys

import numpy as np

sys.path.insert(0, "/opt/trn_rl_repo")

H = 100
N = H * H
NCORES = 8
RANK = 3
FW = RANK * H              # stacked factor width (300)
HOT_B = 4 * H + 2 * FW     # gt f32 | Fstack f16            = 1000 B/row
GCOLD_B = 2 * FW + 4 * (RANK + 1)  # Gstack f16 | gcolcat f32 = 1216 B/row
PCOLD_B = 8 * H            # prob f32 | rowsumN2 f32        =  800 B/row

# PE p-state warmup: free-dim sizes of dummy matmuls issued from t=0.
WARMUP = [256] * 10


def _host_constants():
    """Geometry-only constant tables (input independent)."""
    idx = np.arange(H)
    # rowsum[r,c] = sum_j D[i,j] (i = r*100+c), accumulated in float64.
    absdiff = np.abs(idx[:, None] - idx[None, :])
    q64 = np.sqrt((idx[:, None] ** 2 + idx[None, :] ** 2).astype(np.float64))
    cnt = np.zeros((H, H))
    np.add.at(cnt, (idx[:, None], absdiff), 1.0)
    rowsum = cnt @ q64 @ cnt.T
    # pxrN = (prob>=.5) * rowsumN2 = -2e-8*b for the max trick; the 1e-8
    # (= 1/N^2) output scale is folded into all partial-sum columns so the
    # final step is a plain all-reduce.
    rowsumn2 = (-2.0e-8 * rowsum).astype(np.float32)

    # Pair-count weighted SVD of K[u,v] = sqrt(u^2+v^2), u,v in [-99,99].
    u = np.arange(-(H - 1), H)
    K = np.sqrt((u[:, None] ** 2 + u[None, :] ** 2).astype(np.float64))
    w_cnt = (H - np.abs(u)).astype(np.float64)
    sw = np.sqrt(w_cnt)
    U, S, Vt = np.linalg.svd(sw[:, None] * K * sw[None, :])

    toep = (idx[:, None] - idx[None, :]) + (H - 1)
    fstack = np.empty((H, FW), dtype=np.float16)
    gstack = np.empty((H, FW), dtype=np.float16)
    for s in range(RANK):
        f = (U[:, s] * np.sqrt(S[s])) / sw
        g = (Vt[s, :] * np.sqrt(S[s])) / sw
        # F_s[a, r] = f_s[r-a]; G_s[b, c] = g_s[c-b]  (both symmetric)
        fstack[:, s * H:(s + 1) * H] = f[toep].astype(np.float16)
        gstack[:, s * H:(s + 1) * H] = g[toep].astype(np.float16)
    # gcol_s[b] = 1e-8 * sum_c G16_s[b, c] (sum the f16-rounded table in
    # f64 so the separable sum(a) matches the on-device acc; f32 storage
    # since the 1e-8 prescale would underflow f16)
    gcolcat = np.stack(
        [gstack[:, s * H:(s + 1) * H].astype(np.float64).sum(1)
         for s in range(RANK)], axis=1
    )
    # 4th column: -0.5, turning the fused dot with zrowcat (whose 4th
    # column is the accum of pxrN = -2e-8*b rows) into +1e-8*sum(b)
    gcolcat = np.concatenate(
        [1e-8 * gcolcat, np.full((H, 1), -0.5)], axis=1
    ).astype(np.float32)  # [100, RANK+1]
    return rowsumn2, fstack, gstack, gcolcat


def _build_module():
    import concourse.bacc as bacc
    import concourse.mybir as mybir
    import concourse.tile as tile

    f32 = mybir.dt.float32
    f16 = mybir.dt.float16
    u8 = mybir.dt.uint8
    Alu = mybir.AluOpType

    nc = bacc.Bacc(
        "TRN2",
        target_bir_lowering=False,
        debug=False,
        enable_asserts=False,
        num_devices=NCORES,
    )

    hot_d = nc.dram_tensor("hot", [H, HOT_B], u8, kind="ExternalInput")
    gcold_d = nc.dram_tensor("gcold", [H, GCOLD_B], u8, kind="ExternalInput")
    pcold_d = nc.dram_tensor("pcold", [H, PCOLD_B], u8, kind="ExternalInput")
    out_d = nc.dram_tensor("out", [1, 1], f32, kind="ExternalOutput")

    with tile.TileContext(nc) as tc:
        with (
            tc.tile_pool(name="sb", bufs=1) as sb,
            tc.tile_pool(name="ps_z0", bufs=1, space="PSUM") as ps_z0,
            tc.tile_pool(name="ps_z1", bufs=1, space="PSUM") as ps_z1,
            tc.tile_pool(name="ps_z2", bufs=1, space="PSUM") as ps_z2,
            tc.tile_pool(name="ps_acc", bufs=1, space="PSUM") as ps_acc,
            tc.tile_pool(name="ps_w", bufs=1, space="PSUM") as ps_w,
        ):
            # ---- t=0: three input DMAs on three descriptor queues -------
            # (hot + pcold share SP's HWDGE back-to-back; gcold goes via
            # Pool's SWDGE which generates descriptors on a separate path)
            hot_sb = sb.tile([H, HOT_B], u8)
            nc.sync.dma_start(hot_sb[:], hot_d[:])
            pcold_sb = sb.tile([H, PCOLD_B], u8)
            nc.sync.dma_start(pcold_sb[:], pcold_d[:])
            gcold_sb = sb.tile([H, GCOLD_B], u8)
            nc.gpsimd.dma_start(gcold_sb[:], gcold_d[:])

            gt_v = hot_sb[:, 0:4 * H].bitcast(f32)
            f_v = hot_sb[:, 4 * H:HOT_B].bitcast(f16)
            prob_v = pcold_sb[:, 0:4 * H].bitcast(f32)
            rowsumn2_v = pcold_sb[:, 4 * H:PCOLD_B].bitcast(f32)
            g_v = gcold_sb[:, 0:2 * FW].bitcast(f16)
            gcol_v = gcold_sb[:, 2 * FW:GCOLD_B].bitcast(f32)

            # ---- t=0: constants + PE p-state warmup ---------------------
            wz = sb.tile([H, 256], f16)
            nc.vector.memset(wz[:], 0.0)
            wps = ps_w.tile([H, 256], f32)
            for wn in WARMUP:
                nc.tensor.matmul(
                    wps[:, 0:wn], wz[:, 0:H], wz[:, 0:wn], start=True, stop=True
                )

            # ---- hot path: binarize -> stage1 -> copy -> stage2 ---------
            py16 = sb.tile([H, H], f16)
            nc.vector.tensor_scalar(py16[:], gt_v, 0.5, None, Alu.is_ge)

            zps0 = ps_z0.tile([H, H], f32)
            zps1 = ps_z1.tile([H, H], f32)
            zps2 = ps_z2.tile([H, H], f32)
            for s, zps in enumerate((zps0, zps1, zps2)):
                sl = slice(s * H, (s + 1) * H)
                nc.tensor.matmul(
                    zps[:], py16[:], f_v[:, sl], start=True, stop=True
                )

            # PSUM -> SBUF (fp16) copies, DVE + ACT + DVE (Pool is GPSIMD
            # and cannot read PSUM), DVE ones with a fused per-partition
            # row-sum (zrow_s[b]).
            # (separate tiles: the dep tracker serializes same-tile access)
            zsb0 = sb.tile([H, H], f16)
            zsb1 = sb.tile([H, H], f16)
            zsb2 = sb.tile([H, H], f16)
            # zrow_s[b] = sum_r Z_s[b,r], all three into one DVE-written
            # tile so the downstream product is a single fused op; the 4th
            # column takes the pxrN row-sums (see gcolcat's -0.5 column)
            zrowcat = sb.tile([H, RANK + 1], f32)
            nc.vector.tensor_scalar(
                zsb0[:], zps0[:], 0.0, 0.0, Alu.add, Alu.add,
                accum_out=zrowcat[:, 0:1],
            )
            nc.scalar.copy(zsb1[:], zps1[:])
            nc.vector.tensor_scalar(
                zsb2[:], zps2[:], 0.0, 0.0, Alu.add, Alu.add,
                accum_out=zrowcat[:, 2:3],
            )
            # zrow1 via a cheap 16-bit re-read on DVE (ACT's fused
            # accumulator read would cost a serial 187ns on the zsb1 path)
            junkz = sb.tile([H, H], f16)
            with tc.tile_wait_until(0.0034):
                nc.vector.tensor_scalar(
                    junkz[:], zsb1[:], 0.0, 0.0, Alu.add, Alu.add,
                    accum_out=zrowcat[:, 1:2],
                )

            acc = ps_acc.tile([H, H], f32)
            for s, zsb in enumerate((zsb0, zsb1, zsb2)):
                sl = slice(s * H, (s + 1) * H)
                nc.tensor.matmul(
                    acc[:], zsb[:], g_v[:, sl],
                    start=(s == 0), stop=(s == RANK - 1),
                )

            # ---- tail: partial-sum columns [100, 2], all on DVE ---------
            # (all at 1e-8 = 1/N^2 scale so the finale is a plain reduce;
            # single-engine writers avoid cross-engine same-tile chains)
            # c0: sum(a) + sum(b) rows, one fused dot of zrowcat (zrows +
            #     pxrN row-sums) with gcolcat (1e-8*gcol_s | -0.5)
            # c1: -2e-8*min(a,b) via max(-2e-8*a, pxrN)
            # Pool is GPSIMD: no PSUM access, no general elementwise ops.
            # wait_until hints keep the scheduler from queueing the tail
            # ahead of the critical Z-copies on DVE's in-order queue.
            cols = sb.tile([H, 2], f32)
            pxrn = sb.tile([H, H], f32)
            junkm = sb.tile([H, H], f32)
            with tc.tile_wait_until(0.0035):
                nc.vector.scalar_tensor_tensor(
                    pxrn[:], prob_v, 0.5, rowsumn2_v,
                    op0=Alu.is_ge, op1=Alu.mult,
                    accum_out=zrowcat[:, RANK:RANK + 1],
                )
            junkp = sb.tile([H, RANK + 1], f32)
            with tc.tile_wait_until(0.0037):
                nc.vector.scalar_tensor_tensor(
                    junkp[:], zrowcat[:], 0.0, gcol_v[:],
                    op0=Alu.add, op1=Alu.mult, accum_out=cols[:, 0:1],
                )
            with tc.tile_wait_until(0.0044):
                nc.vector.scalar_tensor_tensor(
                    junkm[:], acc[:], -2.0e-8, pxrn[:],
                    op0=Alu.mult, op1=Alu.max, accum_out=cols[:, 1:2],
                )

            # ---- finale: one Pool XYZWC all-reduce (SBUF-only, legal
            # on GPSIMD; replaces the DVE free-dim reduce + Pool
            # partition_all_reduce pair, saving a cross-engine hop) ------
            res = sb.tile([1, 1], f32)
            nc.gpsimd.tensor_reduce(
                res[:], cols[:], axis=mybir.AxisListType.XYZWC, op=Alu.add
            )
            nc.sync.dma_start(out_d[:], res[:])

    nc.compile()
    return nc


_STATE = {}


def _get_state():
    if not _STATE:
        rowsumn2, fstack, gstack, gcolcat = _host_constants()
        _STATE["consts"] = (rowsumn2, fstack, gstack, gcolcat)
        _STATE["nc"] = _build_module()
    return _STATE


def _in_maps(prob_map, gt_map):
    st = _get_state()
    rowsumn2, fstack, gstack, gcolcat = st["consts"]
    prob = np.asarray(prob_map, dtype=np.float32).reshape(H, H)
    gt = np.asarray(gt_map, dtype=np.float32).reshape(H, H)

    hot = np.ascontiguousarray(np.concatenate(
        [gt.view(np.uint8), fstack.view(np.uint8)], axis=1
    ))
    gcold = np.ascontiguousarray(np.concatenate(
        [gstack.view(np.uint8), gcolcat.view(np.uint8)], axis=1
    ))
    pcold = np.ascontiguousarray(np.concatenate(
        [prob.view(np.uint8), rowsumn2.view(np.uint8)], axis=1
    ))
    return [
        {"hot": hot, "gcold": gcold, "pcold": pcold} for _ in range(NCORES)
    ]


def _run(prob_map, gt_map, trace=False, **spmd_kwargs):
    from concourse import bass_utils

    st = _get_state()
    in_maps = _in_maps(prob_map, gt_map)
    res = bass_utils.run_bass_kernel_spmd(
        st["nc"], in_maps, core_ids=list(range(NCORES)), trace=trace,
        **spmd_kwargs,
    )
    value = np.float32(res.results[0]["out"][0, 0])
    return value, res


def kernel(prob_map, gt_map):
    value, _ = _run(prob_map, gt_map, trace=False)
    return np.asarray(value, dtype=np.float32)
